# revision 29
# baseline (speedup 1.0000x reference)
# BitLinear 1.58 (ternary-weight linear with int8 activation quantization)
# on 8 Trainium2 NeuronCores via Bass/Tile.
#
# Reference computation (fp32):
#   w_scale = max(mean(|W|), 1e-5)           (global over the full weight)
#   W_q     = clip(round(W / w_scale), -1, 1)          (ternary)
#   gamma   = max(max(|x|), 1e-5)            (global over the full activation)
#   x_q     = clip(round(x * 128/gamma), -128, 127)
#   out     = (x_q @ W_q^T) * (gamma*w_scale/128) + bias
#
# Sharding: data-parallel over the 8192 tokens (1024 tokens/core), weight
# replicated. Global scales via per-core partial stats + one tiny AllGather.
#
# Schedule (v2) — changes vs the first working version, driven by the
# perfetto trace (691us):
#  * Stats reads (x shard 16.8MB + w-stats slice 8.4MB) interleaved across
#    BOTH HWDGE rings; per-tile reduces alternate DVE/GpSimd so reduction
#    keeps pace with the DMA stream.
#  * A dummy 8-byte AllGather is issued at kernel start to warm the
#    collective path; the real stats AllGather (measured 34us cold) then
#    runs with the rings/CC core already up, and cross-core launch skew is
#    absorbed during the (DMA-bound) stats window.
#  * PE keep-warm: a dense burst of trivial bf16 matmuls at t~5us promotes
#    the HAM clock gate to 8/8 early, and one paced dummy matmul per stats
#    DMA stops the MID-window re-throttle, so the main loop starts at 2.4GHz.
#  * Main loop: columns 0..1 keep the k-outer/t-inner order (interleaves
#    x-quantize + weight-quantize production with the PE at k granularity);
#    columns 2..7 run token-tile-OUTER sprints (all 32 k-tiles for one
#    PSUM bank back to back) against a full-column wq cache produced one
#    column ahead. Each bank is then evicted ~48us before it is reused, so
#    the PE never waits on PSUM evictions, and the per-column boundary
#    stalls (1-3us each, enough to bounce the HAM clock gate) disappear.
#  * Quantize chains are spread over three engines: ACT does the runtime
#    scale (+magic-round bias), GpSimd does the clip, DVE does the
#    round/pack to bf16. Evictions (x s_o + bias) stay on DVE.
#  * bias replication across partitions via one 0-stride-broadcast DMA.
#
# The matmul contracts over in_features, which must live on the partition
# axis for both operands, so the host pre-transposes x and W once (layout
# prep, outside the device kernel). Quantized operands are fed to the PE in
# bf16 - exact here, because x_q in [-128,127] and W_q in {-1,0,1} are
# integers representable exactly in bf16, and PSUM accumulates in fp32
# (sums bounded by 4096*128 = 2^19 < 2^24, so accumulation is exact).
#
# Rounding: round-half-to-even (= jnp.round) done exactly in fp32 via the
# magic-constant trick (v + 1.5*2^23) - 1.5*2^23, fused into tensor_scalar
# ops. clip-then-round == round-then-clip at these bounds.

import numpy as np
from contextlib import ExitStack

import concourse.bass as bass
import concourse.tile as tile
from concourse import bacc, mybir
from concourse import bass_utils

N_CORES = 8
IN_F = 4096
OUT_F = 4096
TOKENS = 8192  # 4 * 2048
TPC = TOKENS // N_CORES  # tokens per core = 1024
OSL = OUT_F // N_CORES  # per-core weight-stats slice = 512 out_features

KT = IN_F // 128  # 32 k-tiles
CT = OUT_F // 512  # 8 of-columns
TT = TPC // 128  # 8 token-tiles
JIT_COLS = 2  # columns with k-outer/t-inner order (quantize interleave)

MAGIC = 12582912.0  # 1.5 * 2**23: (v + MAGIC) - MAGIC == round-half-even(v)
EPS = 1e-5
F32 = mybir.dt.float32
BF16 = mybir.dt.bfloat16

_cache = {}


def _build():
    nc = bacc.Bacc("TRN2", target_bir_lowering=False, debug=False, num_devices=N_CORES)
    xT = nc.dram_tensor("xT", [IN_F, TPC], F32, kind="ExternalInput").ap()
    wT = nc.dram_tensor("wT", [IN_F, OUT_F], F32, kind="ExternalInput").ap()
    wS = nc.dram_tensor("wS", [IN_F, OSL], F32, kind="ExternalInput").ap()
    bias = nc.dram_tensor("bias", [OUT_F], F32, kind="ExternalInput").ap()
    out = nc.dram_tensor("out", [TPC, OUT_F], F32, kind="ExternalOutput").ap()

    Alu = mybir.AluOpType
    Act = mybir.ActivationFunctionType

    with tile.TileContext(nc) as tc, ExitStack() as ctx:
        ep = ctx.enter_context
        singles = ep(tc.tile_pool(name="singles", bufs=1))
        xin_pool = ep(tc.tile_pool(name="xin", bufs=4))
        spw_pool = ep(tc.tile_pool(name="spw", bufs=2))
        xq_pool = ep(tc.tile_pool(name="xq", bufs=KT))
        win_pool = ep(tc.tile_pool(name="win", bufs=8))
        wq_pool = ep(tc.tile_pool(name="wq", bufs=2 * KT))
        ost_pool = ep(tc.tile_pool(name="ost", bufs=4))
        psum_pool = ep(tc.tile_pool(name="psum", bufs=8, space="PSUM"))
        dram = ep(tc.tile_pool(name="dram", bufs=1, space="DRAM"))

        # ---- tiny constants ----
        ones_row = singles.tile([1, 128], F32)  # for [1,3] scale broadcast matmul
        nc.vector.memset(ones_row[:], 1.0)
        ones_bf = singles.tile([1, 128], BF16)  # PE keep-warm dummies
        nc.vector.memset(ones_bf[:], 1.0)
        zrow_bf = singles.tile([1, 512], BF16)
        nc.vector.memset(zrow_bf[:], 0.0)
        magic_rep = singles.tile([128, TPC], BF16)  # bf16(MAGIC) is exact
        nc.vector.memset(magic_rep[:], MAGIC)

        bias_rep = singles.tile([128, OUT_F], F32)

        # ---- stats phase ----
        # x absmax: 32 tiles [128, TPC] alternating rings; reduce alternating
        # DVE / GpSimd.  w sum|.|: 16 tiles [128, 1024] alternating rings,
        # reduced on ACT (Abs + accumulator).  Ring totals balance at
        # ~12.6MB each.
        SW = 1024
        wrows = SW // OSL  # 2
        NWS = IN_F // (128 * wrows)  # 16
        wv = wS[:].rearrange("(a p x) y -> a p (x y)", p=128, x=wrows)

        xm = singles.tile([128, KT], F32)
        wm = singles.tile([128, NWS], F32)
        stats_dmas = []
        stats_x0 = None
        for k in range(KT):
            # every 4th x tile rides the otherwise-idle SWDGE ring, easing
            # the two saturated HWDGE rings (~12.6MB -> ~10.5MB each)
            if k % 4 == 3:
                eng = nc.gpsimd
            else:
                eng = nc.sync if k % 2 == 0 else nc.scalar
            st = xin_pool.tile([128, TPC], F32, tag="xin", name=f"sx{k}")
            d = eng.dma_start(st[:], xT[k * 128 : (k + 1) * 128, :])
            stats_dmas.append(d)
            if k == 0:
                stats_x0 = st
            nc.vector.tensor_reduce(
                xm[:, k : k + 1], st[:], axis=mybir.AxisListType.X,
                op=Alu.max, apply_absolute_value=True,
            )
            if k % 2 == 1:
                j = k // 2
                wt_ = spw_pool.tile([128, SW], F32, tag="spw", name=f"sw{j}")
                weng = nc.scalar if j % 2 == 0 else nc.sync
                d2 = weng.dma_start(wt_[:], wv[j])
                stats_dmas.append(d2)
                nc.scalar.activation(
                    wt_[:], wt_[:], Act.Abs, accum_out=wm[:, j : j + 1]
                )

        # ---- PE clock ramp: the PE runs at ~2.0GHz until the SW power
        # throttler (~200us firmware loop) has seen sustained activity.
        # The PE is idle during the whole stats+AllGather window anyway, so
        # stream dense realistic matmuls (bf16, K=128, N=512, random x
        # data) through it to finish the ramp before the main loop starts.
        # 420 MMs at ~260ns end around t=110-115us, just before the real
        # first matmul (~140us).
        wu_lhs = singles.tile([128, 128], BF16)
        wu_rhs = singles.tile([128, 512], BF16)
        nc.vector.tensor_copy(wu_lhs[:], stats_x0[:, 0:128])
        nc.vector.tensor_copy(wu_rhs[:], stats_x0[:, 128:640])
        wu_ps = psum_pool.tile([128, 512], F32, tag="ps", name="wu_ps")
        for j in range(420):
            nc.tensor.matmul(wu_ps[:], wu_lhs[:], wu_rhs[:], start=True, stop=True)

        # ---- fold stats and share via one 8B-per-core AllGather ----
        xmax = singles.tile([128, 1], F32)
        nc.vector.tensor_reduce(
            xmax[:], xm[:], axis=mybir.AxisListType.X, op=Alu.max
        )
        wsumc = singles.tile([128, 1], F32)
        nc.vector.tensor_reduce(
            wsumc[:], wm[:], axis=mybir.AxisListType.X, op=Alu.add
        )
        # cross-partition fold directly on GpSimd (no DMA round-trip)
        from concourse import bass_isa

        gx = singles.tile([128, 1], F32)
        nc.gpsimd.partition_all_reduce(
            gx[:], xmax[:], channels=128, reduce_op=bass_isa.ReduceOp.max
        )
        wsum = singles.tile([128, 1], F32)
        nc.gpsimd.partition_all_reduce(
            wsum[:], wsumc[:], channels=128, reduce_op=bass_isa.ReduceOp.add
        )

        cc_sb = singles.tile([1, 2], F32)
        nc.vector.tensor_copy(cc_sb[0:1, 0:1], gx[0:1, :])
        nc.vector.tensor_copy(cc_sb[0:1, 1:2], wsum[0:1, :])
        cc_in = dram.tile([2], F32)
        cc_out = dram.tile([2 * N_CORES], F32)
        cc_in_dma = nc.sync.dma_start(cc_in[:], cc_sb[:])
        # bridge the AllGather window with more clock-ramp matmuls: gated on
        # the collective input send, they run during the >=19us collective
        # latency and keep the HAM clock gate from re-throttling before the
        # first real matmul.
        for j in range(60):
            mm = nc.tensor.matmul(
                wu_ps[:], wu_lhs[:], wu_rhs[:], start=True, stop=True
            )
            if j == 0:
                tile.add_dep_helper(
                    mm.ins, cc_in_dma.ins, sync=True,
                    reason="pace PE keep-warm into the collective window",
                )
        nc.gpsimd.collective_compute(
            "AllGather", Alu.bypass,
            replica_groups=[list(range(N_CORES))],
            ins=[cc_in.opt()], outs=[cc_out.opt()],
        )
        g16 = singles.tile([1, 2 * N_CORES], F32)
        nc.sync.dma_start(g16[:], cc_out[:])
        g3 = g16[:].rearrange("p (r two) -> p two r", two=2)

        # ---- combine gathered stats; per-partition scalar math ----
        gsum = singles.tile([1, 1], F32)
        nc.vector.tensor_reduce(
            gsum[:], g3[0:1, 1:2, :], axis=mybir.AxisListType.X, op=Alu.add
        )
        wscale = singles.tile([1, 1], F32)
        nc.vector.tensor_scalar(
            wscale[:], gsum[:], 1.0 / (OUT_F * IN_F), EPS, Alu.mult, Alu.max
        )
        gmax = singles.tile([1, 1], F32)
        nc.vector.tensor_reduce(
            gmax[:], g3[0:1, 0:1, :], axis=mybir.AxisListType.X, op=Alu.max
        )
        gamma = singles.tile([1, 1], F32)
        nc.vector.tensor_scalar(gamma[:], gmax[:], EPS, None, Alu.max)

        def newton_recip(name, src):
            # correctly-rounded-ish 1/src: HW reciprocal + one Newton step
            r0 = singles.tile([1, 1], F32, tag=f"{name}r0")
            nc.vector.reciprocal(r0[:], src[:])
            t = singles.tile([1, 1], F32, tag=f"{name}t")
            nc.vector.tensor_tensor(t[:], src[:], r0[:], op=Alu.mult)
            u = singles.tile([1, 1], F32, tag=f"{name}u")
            nc.vector.tensor_scalar(u[:], t[:], -1.0, 2.0, Alu.mult, Alu.add)
            r1 = singles.tile([1, 1], F32, tag=f"{name}r1")
            nc.vector.tensor_tensor(r1[:], r0[:], u[:], op=Alu.mult)
            return r1

        rw = newton_recip("rw", wscale)  # 1/w_scale
        rg = newton_recip("rg", gamma)   # 1/gamma
        pack3 = singles.tile([1, 3], F32)
        nc.vector.tensor_scalar(pack3[0:1, 0:1], rg[:], 128.0, None, Alu.mult)
        nc.vector.tensor_copy(pack3[0:1, 1:2], rw[:])
        gws = singles.tile([1, 1], F32)
        nc.vector.tensor_tensor(gws[:], gamma[:], wscale[:], op=Alu.mult)
        nc.vector.tensor_scalar(pack3[0:1, 2:3], gws[:], 2.0 ** -7, None, Alu.mult)
        # broadcast [s_x, r_w, s_o] to all partitions via a K=1 PE matmul
        bp3 = psum_pool.tile([128, 3], F32, tag="ps", name="bp3")
        nc.tensor.matmul(bp3[:], ones_row[:], pack3[:], start=True, stop=True)
        b3 = singles.tile([128, 3], F32)
        nc.vector.tensor_copy(b3[:], bp3[:])
        s_x = b3[:, 0:1]
        r_w = b3[:, 1:2]
        s_o = b3[:, 2:3]

        # ---- bias: replicate across partitions with one 0-stride DMA.
        # Emitted after the stats reads so its 2MB doesn't delay them on
        # ring A; it is only needed by the first eviction (~60us later). ----
        nc.sync.dma_start(bias_rep[:], bias[:].partition_broadcast(128))

        # ---- main loop ----
        xq = [None] * KT

        def emit_xq(k):
            xin = xin_pool.tile([128, TPC], F32, tag="xin", name=f"xin_q{k}")
            nc.sync.dma_start(xin[:], xT[k * 128 : (k + 1) * 128, :])
            # t = x*s_x + MAGIC; the fp32 add rounds t to integer+MAGIC
            # (round-half-even). round(x*s_x) >= -128 always, so only the
            # min-127 side of the clip is needed.  2/3 of the scales go to
            # ACT, 1/3 to DVE, so neither engine gates column 0's pace.
            if k % 3 != 2:
                nc.scalar.activation(
                    xin[:], xin[:], Act.Copy, scale=s_x, bias=MAGIC
                )
            else:
                nc.vector.scalar_tensor_tensor(
                    xin[:], xin[:], s_x, magic_rep[:],
                    op0=Alu.mult, op1=Alu.add,
                )
            xq_k = xq_pool.tile([128, TPC], BF16, tag="xq", name=f"xq{k}")
            nc.vector.tensor_scalar(
                xq_k[:], xin[:], MAGIC, 127.0, Alu.subtract, Alu.min
            )
            xq[k] = xq_k

        def emit_evict(c, t, psum_t):
            of = c * 512
            osb = ost_pool.tile([128, 512], F32, tag="ost", name=f"osb_c{c}_t{t}")
            # out = psum * s_o + bias, one DVE op straight from PSUM
            nc.vector.scalar_tensor_tensor(
                osb[:], psum_t[:], s_o, bias_rep[:, of : of + 512],
                op0=Alu.mult, op1=Alu.add,
            )
            nc.sync.dma_start(out[t * 128 : (t + 1) * 128, of : of + 512], osb[:])

        def emit_wq(c, k):
            of = c * 512
            win = win_pool.tile([128, 512], F32, tag="win", name=f"win_c{c}_k{k}")
            nc.scalar.dma_start(win[:], wT[k * 128 : (k + 1) * 128, of : of + 512])
            nc.scalar.activation(win[:], win[:], Act.Copy, scale=r_w)
            # clip on GpSimd (fp32->fp32 min/max measures ~0.7us there and
            # keeps DVE free); the bf16-converting pack MUST be on DVE —
            # GpSimd takes 7.7us for it.
            nc.gpsimd.tensor_scalar(
                win[:], win[:], 1.0, -1.0, Alu.min, Alu.max
            )
            wq = wq_pool.tile([128, 512], BF16, tag="wq", name=f"wq_c{c}_k{k}")
            nc.vector.tensor_scalar(
                wq[:], win[:], MAGIC, MAGIC, Alu.add, Alu.subtract
            )
            return wq

        for c in range(CT):
            wqs = []
            psums = None
            for k in range(KT):
                if c == 0:
                    emit_xq(k)
                wqs.append(emit_wq(c, k))
                if c < JIT_COLS:
                    if k == 0:
                        psums = [
                            psum_pool.tile(
                                [128, 512], F32, tag="ps", name=f"ps_c{c}_t{t}"
                            )
                            for t in range(TT)
                        ]
                    for t in range(TT):
                        nc.tensor.matmul(
                            psums[t][:], xq[k][:, t * 128 : (t + 1) * 128],
                            wqs[k][:], start=(k == 0), stop=(k == KT - 1),
                        )
            if c < JIT_COLS:
                for t in range(TT):
                    emit_evict(c, t, psums[t])
            else:
                # token-tile-outer sprints against the cached wq column:
                # bank t is freed (evicted) ~7 sprints before it is reused.
                for t in range(TT):
                    ps = psum_pool.tile(
                        [128, 512], F32, tag="ps", name=f"ps_c{c}_t{t}"
                    )
                    for k in range(KT):
                        nc.tensor.matmul(
                            ps[:], xq[k][:, t * 128 : (t + 1) * 128],
                            wqs[k][:], start=(k == 0), stop=(k == KT - 1),
                        )
                    emit_evict(c, t, ps)

    nc.compile()
    return nc


def _prep_inputs(x, weight, bias):
    x2 = np.ascontiguousarray(x.reshape(TOKENS, IN_F).T)  # [IN_F, TOKENS]
    wT = np.ascontiguousarray(weight.T)  # [IN_F, OUT_F]
    in_maps = []
    for i in range(N_CORES):
        in_maps.append(
            {
                "xT": np.ascontiguousarray(x2[:, i * TPC : (i + 1) * TPC]),
                "wT": wT,
                "wS": np.ascontiguousarray(wT[:, i * OSL : (i + 1) * OSL]),
                "bias": bias,
            }
        )
    return in_maps


def _run(x, weight, bias, trace=False):
    if "nc" not in _cache:
        _cache["nc"] = _build()
    nc = _cache["nc"]
    in_maps = _prep_inputs(
        np.asarray(x, dtype=np.float32),
        np.asarray(weight, dtype=np.float32),
        np.asarray(bias, dtype=np.float32),
    )
    res = bass_utils.run_bass_kernel_spmd(
        nc, in_maps, list(range(N_CORES)), trace=trace
    )
    full = np.concatenate(
        [res.results[i]["out"] for i in range(N_CORES)], axis=0
    )
    return full.reshape(4, 2048, OUT_F), res


def kernel(x, weight, bias):
    out, _ = _run(x, weight, bias)
    return out


# revision 30
# speedup vs baseline: 1.0058x; 1.0058x over previous
# BitLinear 1.58 (ternary-weight linear with int8 activation quantization)
# on 8 Trainium2 NeuronCores via Bass/Tile.
#
# Reference computation (fp32):
#   w_scale = max(mean(|W|), 1e-5)           (global over the full weight)
#   W_q     = clip(round(W / w_scale), -1, 1)          (ternary)
#   gamma   = max(max(|x|), 1e-5)            (global over the full activation)
#   x_q     = clip(round(x * 128/gamma), -128, 127)
#   out     = (x_q @ W_q^T) * (gamma*w_scale/128) + bias
#
# Sharding: data-parallel over the 8192 tokens (1024 tokens/core), weight
# replicated. Global scales via per-core partial stats + one tiny AllGather.
#
# Schedule (v2) — changes vs the first working version, driven by the
# perfetto trace (691us):
#  * Stats reads (x shard 16.8MB + w-stats slice 8.4MB) interleaved across
#    BOTH HWDGE rings; per-tile reduces alternate DVE/GpSimd so reduction
#    keeps pace with the DMA stream.
#  * A dummy 8-byte AllGather is issued at kernel start to warm the
#    collective path; the real stats AllGather (measured 34us cold) then
#    runs with the rings/CC core already up, and cross-core launch skew is
#    absorbed during the (DMA-bound) stats window.
#  * PE keep-warm: a dense burst of trivial bf16 matmuls at t~5us promotes
#    the HAM clock gate to 8/8 early, and one paced dummy matmul per stats
#    DMA stops the MID-window re-throttle, so the main loop starts at 2.4GHz.
#  * Main loop: columns 0..1 keep the k-outer/t-inner order (interleaves
#    x-quantize + weight-quantize production with the PE at k granularity);
#    columns 2..7 run token-tile-OUTER sprints (all 32 k-tiles for one
#    PSUM bank back to back) against a full-column wq cache produced one
#    column ahead. Each bank is then evicted ~48us before it is reused, so
#    the PE never waits on PSUM evictions, and the per-column boundary
#    stalls (1-3us each, enough to bounce the HAM clock gate) disappear.
#  * Quantize chains are spread over three engines: ACT does the runtime
#    scale (+magic-round bias), GpSimd does the clip, DVE does the
#    round/pack to bf16. Evictions (x s_o + bias) stay on DVE.
#  * bias replication across partitions via one 0-stride-broadcast DMA.
#
# The matmul contracts over in_features, which must live on the partition
# axis for both operands, so the host pre-transposes x and W once (layout
# prep, outside the device kernel). Quantized operands are fed to the PE in
# bf16 - exact here, because x_q in [-128,127] and W_q in {-1,0,1} are
# integers representable exactly in bf16, and PSUM accumulates in fp32
# (sums bounded by 4096*128 = 2^19 < 2^24, so accumulation is exact).
#
# Rounding: round-half-to-even (= jnp.round) done exactly in fp32 via the
# magic-constant trick (v + 1.5*2^23) - 1.5*2^23, fused into tensor_scalar
# ops. clip-then-round == round-then-clip at these bounds.

import numpy as np
from contextlib import ExitStack

import concourse.bass as bass
import concourse.tile as tile
from concourse import bacc, mybir
from concourse import bass_utils

N_CORES = 8
IN_F = 4096
OUT_F = 4096
TOKENS = 8192  # 4 * 2048
TPC = TOKENS // N_CORES  # tokens per core = 1024
OSL = OUT_F // N_CORES  # per-core weight-stats slice = 512 out_features

KT = IN_F // 128  # 32 k-tiles
CT = OUT_F // 512  # 8 of-columns
TT = TPC // 128  # 8 token-tiles
JIT_COLS = 2  # columns with k-outer/t-inner order (quantize interleave)

MAGIC = 12582912.0  # 1.5 * 2**23: (v + MAGIC) - MAGIC == round-half-even(v)
EPS = 1e-5
F32 = mybir.dt.float32
BF16 = mybir.dt.bfloat16

_cache = {}


def _build():
    nc = bacc.Bacc("TRN2", target_bir_lowering=False, debug=False, num_devices=N_CORES)
    xT = nc.dram_tensor("xT", [IN_F, TPC], F32, kind="ExternalInput").ap()
    wT = nc.dram_tensor("wT", [IN_F, OUT_F], F32, kind="ExternalInput").ap()
    wS = nc.dram_tensor("wS", [IN_F, OSL], F32, kind="ExternalInput").ap()
    bias = nc.dram_tensor("bias", [OUT_F], F32, kind="ExternalInput").ap()
    out = nc.dram_tensor("out", [TPC, OUT_F], F32, kind="ExternalOutput").ap()

    Alu = mybir.AluOpType
    Act = mybir.ActivationFunctionType

    with tile.TileContext(nc) as tc, ExitStack() as ctx:
        ep = ctx.enter_context
        singles = ep(tc.tile_pool(name="singles", bufs=1))
        xin_pool = ep(tc.tile_pool(name="xin", bufs=4))
        spw_pool = ep(tc.tile_pool(name="spw", bufs=2))
        xq_pool = ep(tc.tile_pool(name="xq", bufs=KT))
        win_pool = ep(tc.tile_pool(name="win", bufs=8))
        wq_pool = ep(tc.tile_pool(name="wq", bufs=2 * KT))
        ost_pool = ep(tc.tile_pool(name="ost", bufs=4))
        psum_pool = ep(tc.tile_pool(name="psum", bufs=8, space="PSUM"))
        dram = ep(tc.tile_pool(name="dram", bufs=1, space="DRAM"))

        # ---- tiny constants ----
        ones_row = singles.tile([1, 128], F32)  # for [1,3] scale broadcast matmul
        nc.vector.memset(ones_row[:], 1.0)
        ones_bf = singles.tile([1, 128], BF16)  # PE keep-warm dummies
        nc.vector.memset(ones_bf[:], 1.0)
        zrow_bf = singles.tile([1, 512], BF16)
        nc.vector.memset(zrow_bf[:], 0.0)
        magic_rep = singles.tile([128, TPC], BF16)  # bf16(MAGIC) is exact
        nc.vector.memset(magic_rep[:], MAGIC)

        bias_rep = singles.tile([128, OUT_F], F32)

        # ---- stats phase ----
        # x absmax: 32 tiles [128, TPC] alternating rings; reduce alternating
        # DVE / GpSimd.  w sum|.|: 16 tiles [128, 1024] alternating rings,
        # reduced on ACT (Abs + accumulator).  Ring totals balance at
        # ~12.6MB each.
        SW = 1024
        wrows = SW // OSL  # 2
        NWS = IN_F // (128 * wrows)  # 16
        wv = wS[:].rearrange("(a p x) y -> a p (x y)", p=128, x=wrows)

        xm = singles.tile([128, KT], F32)
        wm = singles.tile([128, NWS], F32)
        stats_dmas = []
        stats_x0 = None
        for k in range(KT):
            # every 4th x tile rides the otherwise-idle SWDGE ring, easing
            # the two saturated HWDGE rings (~12.6MB -> ~10.5MB each)
            if k % 4 == 3:
                eng = nc.gpsimd
            else:
                eng = nc.sync if k % 2 == 0 else nc.scalar
            st = xin_pool.tile([128, TPC], F32, tag="xin", name=f"sx{k}")
            d = eng.dma_start(st[:], xT[k * 128 : (k + 1) * 128, :])
            stats_dmas.append(d)
            if k == 0:
                stats_x0 = st
            nc.vector.tensor_reduce(
                xm[:, k : k + 1], st[:], axis=mybir.AxisListType.X,
                op=Alu.max, apply_absolute_value=True,
            )
            if k % 2 == 1:
                j = k // 2
                wt_ = spw_pool.tile([128, SW], F32, tag="spw", name=f"sw{j}")
                weng = nc.scalar if j % 2 == 0 else nc.sync
                d2 = weng.dma_start(wt_[:], wv[j])
                stats_dmas.append(d2)
                nc.scalar.activation(
                    wt_[:], wt_[:], Act.Abs, accum_out=wm[:, j : j + 1]
                )

        # ---- PE clock ramp: the PE runs at ~2.0GHz until the SW power
        # throttler (~200us firmware loop) has seen sustained activity.
        # The PE is idle during the whole stats+AllGather window anyway, so
        # stream dense realistic matmuls (bf16, K=128, N=512, random x
        # data) through it to finish the ramp before the main loop starts.
        # 420 MMs at ~260ns end around t=110-115us, just before the real
        # first matmul (~140us).
        wu_lhs = singles.tile([128, 128], BF16)
        wu_rhs = singles.tile([128, 512], BF16)
        nc.vector.tensor_copy(wu_lhs[:], stats_x0[:, 0:128])
        nc.vector.tensor_copy(wu_rhs[:], stats_x0[:, 128:640])
        wu_ps = psum_pool.tile([128, 512], F32, tag="ps", name="wu_ps")
        for j in range(420):
            nc.tensor.matmul(wu_ps[:], wu_lhs[:], wu_rhs[:], start=True, stop=True)

        # ---- fold stats and share via one 8B-per-core AllGather ----
        xmax = singles.tile([128, 1], F32)
        nc.vector.tensor_reduce(
            xmax[:], xm[:], axis=mybir.AxisListType.X, op=Alu.max
        )
        wsumc = singles.tile([128, 1], F32)
        nc.vector.tensor_reduce(
            wsumc[:], wm[:], axis=mybir.AxisListType.X, op=Alu.add
        )
        # cross-partition fold directly on GpSimd (no DMA round-trip)
        from concourse import bass_isa

        gx = singles.tile([128, 1], F32)
        nc.gpsimd.partition_all_reduce(
            gx[:], xmax[:], channels=128, reduce_op=bass_isa.ReduceOp.max
        )
        wsum = singles.tile([128, 1], F32)
        nc.gpsimd.partition_all_reduce(
            wsum[:], wsumc[:], channels=128, reduce_op=bass_isa.ReduceOp.add
        )

        cc_sb = singles.tile([1, 2], F32)
        nc.vector.tensor_copy(cc_sb[0:1, 0:1], gx[0:1, :])
        nc.vector.tensor_copy(cc_sb[0:1, 1:2], wsum[0:1, :])
        cc_in = dram.tile([2], F32)
        cc_out = dram.tile([2 * N_CORES], F32)
        cc_in_dma = nc.sync.dma_start(cc_in[:], cc_sb[:])
        # bridge the AllGather window with more clock-ramp matmuls: gated on
        # the collective input send, they keep the HAM clock gate from
        # re-throttling before the first real matmul.  They queue behind the
        # 420-stream (ends ~125us) while the first real matmul can be ready
        # at cc_in+24us (~131us) when the collective is fast, so the count
        # is capped at ~6.5us worth to never delay the real work.
        for j in range(25):
            mm = nc.tensor.matmul(
                wu_ps[:], wu_lhs[:], wu_rhs[:], start=True, stop=True
            )
            if j == 0:
                tile.add_dep_helper(
                    mm.ins, cc_in_dma.ins, sync=True,
                    reason="pace PE keep-warm into the collective window",
                )
        nc.gpsimd.collective_compute(
            "AllGather", Alu.bypass,
            replica_groups=[list(range(N_CORES))],
            ins=[cc_in.opt()], outs=[cc_out.opt()],
        )
        g16 = singles.tile([1, 2 * N_CORES], F32)
        nc.sync.dma_start(g16[:], cc_out[:])
        g3 = g16[:].rearrange("p (r two) -> p two r", two=2)

        # ---- combine gathered stats; per-partition scalar math ----
        gsum = singles.tile([1, 1], F32)
        nc.vector.tensor_reduce(
            gsum[:], g3[0:1, 1:2, :], axis=mybir.AxisListType.X, op=Alu.add
        )
        wscale = singles.tile([1, 1], F32)
        nc.vector.tensor_scalar(
            wscale[:], gsum[:], 1.0 / (OUT_F * IN_F), EPS, Alu.mult, Alu.max
        )
        gmax = singles.tile([1, 1], F32)
        nc.vector.tensor_reduce(
            gmax[:], g3[0:1, 0:1, :], axis=mybir.AxisListType.X, op=Alu.max
        )
        gamma = singles.tile([1, 1], F32)
        nc.vector.tensor_scalar(gamma[:], gmax[:], EPS, None, Alu.max)

        def newton_recip(name, src):
            # correctly-rounded-ish 1/src: HW reciprocal + one Newton step
            r0 = singles.tile([1, 1], F32, tag=f"{name}r0")
            nc.vector.reciprocal(r0[:], src[:])
            t = singles.tile([1, 1], F32, tag=f"{name}t")
            nc.vector.tensor_tensor(t[:], src[:], r0[:], op=Alu.mult)
            u = singles.tile([1, 1], F32, tag=f"{name}u")
            nc.vector.tensor_scalar(u[:], t[:], -1.0, 2.0, Alu.mult, Alu.add)
            r1 = singles.tile([1, 1], F32, tag=f"{name}r1")
            nc.vector.tensor_tensor(r1[:], r0[:], u[:], op=Alu.mult)
            return r1

        rw = newton_recip("rw", wscale)  # 1/w_scale
        rg = newton_recip("rg", gamma)   # 1/gamma
        pack3 = singles.tile([1, 3], F32)
        nc.vector.tensor_scalar(pack3[0:1, 0:1], rg[:], 128.0, None, Alu.mult)
        nc.vector.tensor_copy(pack3[0:1, 1:2], rw[:])
        gws = singles.tile([1, 1], F32)
        nc.vector.tensor_tensor(gws[:], gamma[:], wscale[:], op=Alu.mult)
        nc.vector.tensor_scalar(pack3[0:1, 2:3], gws[:], 2.0 ** -7, None, Alu.mult)
        # broadcast [s_x, r_w, s_o] to all partitions via a K=1 PE matmul
        bp3 = psum_pool.tile([128, 3], F32, tag="ps", name="bp3")
        nc.tensor.matmul(bp3[:], ones_row[:], pack3[:], start=True, stop=True)
        b3 = singles.tile([128, 3], F32)
        nc.vector.tensor_copy(b3[:], bp3[:])
        s_x = b3[:, 0:1]
        r_w = b3[:, 1:2]
        s_o = b3[:, 2:3]

        # ---- bias: replicate across partitions with one 0-stride DMA.
        # Emitted after the stats reads so its 2MB doesn't delay them on
        # ring A; it is only needed by the first eviction (~60us later). ----
        nc.sync.dma_start(bias_rep[:], bias[:].partition_broadcast(128))

        # ---- main loop ----
        xq = [None] * KT

        def emit_xq(k):
            xin = xin_pool.tile([128, TPC], F32, tag="xin", name=f"xin_q{k}")
            nc.sync.dma_start(xin[:], xT[k * 128 : (k + 1) * 128, :])
            # t = x*s_x + MAGIC; the fp32 add rounds t to integer+MAGIC
            # (round-half-even). round(x*s_x) >= -128 always, so only the
            # min-127 side of the clip is needed.  2/3 of the scales go to
            # ACT, 1/3 to DVE, so neither engine gates column 0's pace.
            if k % 3 != 2:
                nc.scalar.activation(
                    xin[:], xin[:], Act.Copy, scale=s_x, bias=MAGIC
                )
            else:
                nc.vector.scalar_tensor_tensor(
                    xin[:], xin[:], s_x, magic_rep[:],
                    op0=Alu.mult, op1=Alu.add,
                )
            xq_k = xq_pool.tile([128, TPC], BF16, tag="xq", name=f"xq{k}")
            nc.vector.tensor_scalar(
                xq_k[:], xin[:], MAGIC, 127.0, Alu.subtract, Alu.min
            )
            xq[k] = xq_k

        def emit_evict(c, t, psum_t):
            of = c * 512
            osb = ost_pool.tile([128, 512], F32, tag="ost", name=f"osb_c{c}_t{t}")
            # out = psum * s_o + bias, one DVE op straight from PSUM
            nc.vector.scalar_tensor_tensor(
                osb[:], psum_t[:], s_o, bias_rep[:, of : of + 512],
                op0=Alu.mult, op1=Alu.add,
            )
            nc.sync.dma_start(out[t * 128 : (t + 1) * 128, of : of + 512], osb[:])

        def emit_wq(c, k):
            of = c * 512
            win = win_pool.tile([128, 512], F32, tag="win", name=f"win_c{c}_k{k}")
            nc.scalar.dma_start(win[:], wT[k * 128 : (k + 1) * 128, of : of + 512])
            nc.scalar.activation(win[:], win[:], Act.Copy, scale=r_w)
            # clip on GpSimd (fp32->fp32 min/max measures ~0.7us there and
            # keeps DVE free); the bf16-converting pack MUST be on DVE —
            # GpSimd takes 7.7us for it.
            nc.gpsimd.tensor_scalar(
                win[:], win[:], 1.0, -1.0, Alu.min, Alu.max
            )
            wq = wq_pool.tile([128, 512], BF16, tag="wq", name=f"wq_c{c}_k{k}")
            nc.vector.tensor_scalar(
                wq[:], win[:], MAGIC, MAGIC, Alu.add, Alu.subtract
            )
            return wq

        for c in range(CT):
            wqs = []
            psums = None
            for k in range(KT):
                if c == 0:
                    emit_xq(k)
                wqs.append(emit_wq(c, k))
                if c < JIT_COLS:
                    if k == 0:
                        psums = [
                            psum_pool.tile(
                                [128, 512], F32, tag="ps", name=f"ps_c{c}_t{t}"
                            )
                            for t in range(TT)
                        ]
                    for t in range(TT):
                        nc.tensor.matmul(
                            psums[t][:], xq[k][:, t * 128 : (t + 1) * 128],
                            wqs[k][:], start=(k == 0), stop=(k == KT - 1),
                        )
            if c < JIT_COLS:
                for t in range(TT):
                    emit_evict(c, t, psums[t])
            else:
                # token-tile-outer sprints against the cached wq column:
                # bank t is freed (evicted) ~7 sprints before it is reused.
                for t in range(TT):
                    ps = psum_pool.tile(
                        [128, 512], F32, tag="ps", name=f"ps_c{c}_t{t}"
                    )
                    for k in range(KT):
                        nc.tensor.matmul(
                            ps[:], xq[k][:, t * 128 : (t + 1) * 128],
                            wqs[k][:], start=(k == 0), stop=(k == KT - 1),
                        )
                    emit_evict(c, t, ps)

    nc.compile()
    return nc


def _prep_inputs(x, weight, bias):
    x2 = np.ascontiguousarray(x.reshape(TOKENS, IN_F).T)  # [IN_F, TOKENS]
    wT = np.ascontiguousarray(weight.T)  # [IN_F, OUT_F]
    in_maps = []
    for i in range(N_CORES):
        in_maps.append(
            {
                "xT": np.ascontiguousarray(x2[:, i * TPC : (i + 1) * TPC]),
                "wT": wT,
                "wS": np.ascontiguousarray(wT[:, i * OSL : (i + 1) * OSL]),
                "bias": bias,
            }
        )
    return in_maps


def _run(x, weight, bias, trace=False):
    if "nc" not in _cache:
        _cache["nc"] = _build()
    nc = _cache["nc"]
    in_maps = _prep_inputs(
        np.asarray(x, dtype=np.float32),
        np.asarray(weight, dtype=np.float32),
        np.asarray(bias, dtype=np.float32),
    )
    res = bass_utils.run_bass_kernel_spmd(
        nc, in_maps, list(range(N_CORES)), trace=trace
    )
    full = np.concatenate(
        [res.results[i]["out"] for i in range(N_CORES)], axis=0
    )
    return full.reshape(4, 2048, OUT_F), res


def kernel(x, weight, bias):
    out, _ = _run(x, weight, bias)
    return out
